# revision 8
# baseline (speedup 1.0000x reference)
"""Trainium2 Bass kernel for nn_ClusterEncoder (PointTransformerConv-style
GNN message passing), 8-core SPMD.

Strategy (edges sharded by destination node; fp16 data plane):
  * Host: sort edges by dst, split nodes into 8 equal contiguous ranges
    (edge counts balance to ~0.3% for this random graph). Within a core,
    greedy-pack destination nodes into "chunks" of <=128 nodes and
    <=CHUNK_E edges; pad each chunk's edge list to CHUNK_E slots.
    Each core receives ONLY its node shard (xT, fp16, transposed) plus
    its edge maps, consolidated into 5 arrays (~3.3 MB/core instead of
    a replicated 25.6 MB x): per-edge dst ids, local chunk-row ids and
    per-chunk output rows ride in one packed int32 array.
  * Device, phase 1 (local shard only): U_loc = x_c @ (W_dst@Wa1) and
    VH_loc = x_c @ [W_src@Wa1 | W_lin] for the core's own 6250 nodes.
  * AllGather VH_loc across the 8 cores -> vh_full [N, 192] fp16
    (contiguous node shards concatenate rank-major, so global src ids
    index it directly). U stays local: dst ids are core-local by the
    edge sharding, so the U gather reads the local table.
  * Device, phase 2 (per chunk of 16 x 128-edge tiles):
      - gather VH rows by src (384B/row) and U rows by local dst,
      - pos MLP: t_p1 = relu(Wp1^T posd^T + bp1), delta = relu(Wp2^T t_p1 + bp2),
      - z1 = Wa1^T delta;  t_a = relu(z1 + (U[dst]-V[src])^T + ba1),
        with the per-tile (U-V)^T transposes accumulated straight into
        the z1 PSUM group,
      - logits = relu(Wa2^T t_a + ba2);  e = exp(logits - SHIFT)
        (softmax max-subtraction replaced by a constant shift -- exactly
        equivalent math since the shift cancels in e/sum(e); logits are
        relu-bounded so no overflow),
      - one-hot indicator per tile from local dst index (iota + is_equal),
      - segment-sum via matmul: acc[n, 0:128] += ind^T @ (e*(H[src]+delta))^T,
        acc[n, 128:256] += ind^T @ e^T   (numerator and normalizer together),
      - out = relu(NUM / (s + eps)); indirect-scatter fp16 rows to y.
  * Softmax segments are core-local by construction, so the only
    collective is the single VH AllGather.
"""
import sys
from dataclasses import dataclass
from math import ceil

if "/opt/trn_rl_repo" not in sys.path:
    sys.path.insert(0, "/opt/trn_rl_repo")

import numpy as np

import concourse.bass as bass
import concourse.mybir as mybir
import concourse.tile as tile
from concourse import bacc
from concourse.bass import IndirectOffsetOnAxis, ts
from concourse.bass_utils import run_bass_kernel_spmd
from concourse.masks import make_identity

f32 = mybir.dt.float32
f16 = mybir.dt.float16
i32 = mybir.dt.int32
AF = mybir.ActivationFunctionType
ALU = mybir.AluOpType


@dataclass
class Cfg:
    N: int = 50000
    C: int = 128
    PH: int = 64
    AH: int = 64
    DIM: int = 2
    M: int = 8            # cores
    T: int = 16           # 128-edge tiles per chunk
    TB: int = 4           # tiles per matmul block (block = 512 edges)
    SHIFT: float = 8.0
    EPS: float = 1e-12

    @property
    def NLOC(self):
        return self.N // self.M

    @property
    def CHUNK_E(self):
        return self.T * 128

    @property
    def OUT_ROWS(self):
        return self.NLOC + 1  # +1 trash row for padded scatter lanes


CFG = Cfg()

# wpack column layout (fp16 [128, WCOLS])
WC_NODE = 0          # [0:128, 0:256]   Wda | Wsa | W_lin
WC_P1 = 256          # [0:2,   256:320] Wp1
WC_P2 = 320          # [0:64,  320:448] Wp2
WC_A1 = 448          # [0:128, 448:512] Wa1
WC_A2 = 512          # [0:64,  512:640] Wa2
WC_B = 640           # [0:128, 640:645] bp1 | bp2 | ba1 | ba2 | -SHIFT
WCOLS = 648


# ---------------------------------------------------------------- host pack
def _pack(x, pos, edge_index, cfg):
    """Sort/shard/chunk edges; returns per-core input dicts (minus weights)."""
    src = np.asarray(edge_index[0], np.int64)
    dst = np.asarray(edge_index[1], np.int64)
    order = np.argsort(dst, kind="stable")
    s_s = src[order]
    d_s = dst[order]
    posd = (pos[d_s] - pos[s_s]).astype(np.float16)  # [E, 2]

    NLOC = cfg.NLOC
    bounds = np.searchsorted(d_s, np.arange(cfg.M + 1) * NLOC)

    cores = []
    for c in range(cfg.M):
        lo, hi = bounds[c], bounds[c + 1]
        dloc = d_s[lo:hi] - c * NLOC
        deg = np.bincount(dloc, minlength=NLOC)
        nodes = np.nonzero(deg)[0]
        chunks = []  # (node_list, e0, e1) ; e relative to lo
        cur, cur_e, estart = [], 0, 0
        for n in nodes:
            dn = int(deg[n])
            assert dn <= cfg.CHUNK_E, f"degree {dn} exceeds chunk capacity"
            if len(cur) == 128 or cur_e + dn > cfg.CHUNK_E:
                chunks.append((cur, estart, estart + cur_e))
                estart += cur_e
                cur, cur_e = [], 0
            cur.append(int(n))
            cur_e += dn
        if cur:
            chunks.append((cur, estart, estart + cur_e))
        cores.append((lo, chunks, dloc))

    NCHUNK = max(len(ch) for _, ch, _ in cores) if cores else 1
    NCHUNK = max(NCHUNK, 1)

    in_maps = []
    for c in range(cfg.M):
        lo, chunks, dloc = cores[c]
        srcid = np.zeros((NCHUNK, 128, cfg.T), np.int32)
        # emap[..., :T] = local dst id | (local chunk row + 1) << 13
        # emap[..., T]  = per-chunk output rows (trash row NLOC for pads)
        emap = np.zeros((NCHUNK, 128, cfg.T + 1), np.int32)
        emap[:, :, cfg.T] = cfg.NLOC
        posdT = np.zeros((NCHUNK, cfg.DIM, cfg.CHUNK_E), np.float16)
        for k, (nl, e0, e1) in enumerate(chunks):
            cnt = e1 - e0
            g0, g1 = lo + e0, lo + e1
            nla = np.asarray(nl, np.int64)
            loc = np.searchsorted(nla, dloc[e0:e1]).astype(np.int32)
            j = np.arange(cnt)
            t_idx = j >> 7
            lane = j & 127
            srcid[k, lane, t_idx] = s_s[g0:g1].astype(np.int32)
            emap[k, lane, t_idx] = (dloc[e0:e1].astype(np.int32)
                                    | ((loc + 1) << 13))
            posdT[k, :, :cnt] = posd[g0:g1].T
            emap[k, : len(nl), cfg.T] = nla.astype(np.int32)
        in_maps.append(dict(srcid=srcid, emap=emap, posdT=posdT))
    return in_maps, NCHUNK


# ---------------------------------------------------------------- program
def _build(cfg, nchunk):
    nc = bacc.Bacc(None, target_bir_lowering=False, num_devices=cfg.M)
    N, C, PH, AH, DIM = cfg.N, cfg.C, cfg.PH, cfg.AH, cfg.DIM
    NLOC = cfg.NLOC

    xT_d = nc.declare_dram_parameter("xT", [C, NLOC], f16, isOutput=False)
    wpack_d = nc.declare_dram_parameter("wpack", [128, WCOLS], f16, isOutput=False)
    src_d = nc.declare_dram_parameter("srcid", [nchunk * 128, cfg.T], i32, isOutput=False)
    em_d = nc.declare_dram_parameter("emap", [nchunk * 128, cfg.T + 1], i32, isOutput=False)
    pd_d = nc.declare_dram_parameter("posdT", [nchunk * DIM, cfg.CHUNK_E], f16, isOutput=False)
    y_d = nc.declare_dram_parameter("y", [cfg.OUT_ROWS, C], f16, isOutput=True)

    U_loc = nc.dram_tensor("U_loc", [cfg.OUT_ROWS, AH], f16)  # x_c @ (W_dst@Wa1)
    vh_send = nc.dram_tensor("vh_send", [NLOC, AH + C], f16)  # x_c @ [W_src@Wa1 | W_lin]
    vh_full = nc.dram_tensor("vh_full", [N, AH + C], f16, addr_space="Shared")

    NB = cfg.T // cfg.TB  # blocks per chunk
    BLK = cfg.TB * 128

    with tile.TileContext(nc) as tc:
        with tc.tile_pool(name="const", bufs=1) as cp:
            wpack_s = cp.tile([128, WCOLS], f16)
            nc.sync.dma_start(out=wpack_s[:], in_=wpack_d[:, :])
            wnode_s = wpack_s[:, WC_NODE:WC_NODE + 2 * AH + C]
            wp1_s = wpack_s[0:DIM, WC_P1:WC_P1 + PH]
            wp2_s = wpack_s[0:PH, WC_P2:WC_P2 + C]
            wa1_s = wpack_s[:, WC_A1:WC_A1 + AH]
            wa2_s = wpack_s[0:AH, WC_A2:WC_A2 + C]
            bp1_b = wpack_s[0:PH, WC_B + 0:WC_B + 1]
            bp2_b = wpack_s[:, WC_B + 1:WC_B + 2]
            ba1_b = wpack_s[0:AH, WC_B + 2:WC_B + 3]
            ba2_b = wpack_s[:, WC_B + 3:WC_B + 4]
            shift_b = wpack_s[:, WC_B + 4:WC_B + 5]
            ident_s = cp.tile([128, 128], f16)
            make_identity(nc, ident_s[:])
            ident32_s = cp.tile([128, 128], f32)
            make_identity(nc, ident32_s[:])
            iota_i = cp.tile([128, 128], i32)
            nc.gpsimd.iota(iota_i[:], pattern=[[1, 128]], base=1, channel_multiplier=0)
            iota_s = cp.tile([128, 128], f16)
            nc.vector.tensor_copy(iota_s[:], iota_i[:])
            xT_s = cp.tile([C, NLOC], f16)
            nc.sync.dma_start(out=xT_s[:], in_=xT_d[:, :])

            # ---------------- phase 1: local node features U / VH ----------
            with tc.tile_pool(name="p1", bufs=3) as p1, \
                 tc.tile_pool(name="p1ps", bufs=2, space="PSUM") as p1ps:
                zr_s = p1.tile([1, AH], f16, tag="zr")
                nc.gpsimd.memset(zr_s[:], 0.0)
                nc.sync.dma_start(out=U_loc[NLOC:NLOC + 1, :], in_=zr_s[:])
                nt = ceil(NLOC / 128)
                for t in range(nt):
                    r0 = t * 128
                    rows = min(128, NLOC - r0)
                    uvh_p = p1ps.tile([128, 2 * AH + C], f32, tag="uvh")
                    nc.tensor.matmul(uvh_p[:rows, :], lhsT=xT_s[:, r0:r0 + rows],
                                     rhs=wnode_s, start=True, stop=True)
                    uvh_s = p1.tile([128, 2 * AH + C], f16, tag="uvhs")
                    nc.scalar.activation(uvh_s[:rows, :], uvh_p[:rows, :], AF.Copy)
                    nc.sync.dma_start(out=U_loc[r0:r0 + rows, :], in_=uvh_s[:rows, 0:AH])
                    nc.sync.dma_start(out=vh_send[r0:r0 + rows, :], in_=uvh_s[:rows, AH:])

            # ---------------- all-gather VH across cores ----------
            nc.gpsimd.collective_compute(
                "AllGather",
                mybir.AluOpType.bypass,
                replica_groups=[list(range(cfg.M))],
                ins=[vh_send[:, :]],
                outs=[vh_full[:, :]],
            )

            # ---------------- phase 2: edges ----------------
            with tc.tile_pool(name="eb", bufs=3) as eb, \
                 tc.tile_pool(name="ebg", bufs=3) as ebg, \
                 tc.tile_pool(name="ps_acc", bufs=2, space="PSUM") as ps_acc, \
                 tc.tile_pool(name="ps_b", bufs=1, space="PSUM") as ps_b, \
                 tc.tile_pool(name="ps_c", bufs=1, space="PSUM") as ps_c, \
                 tc.tile_pool(name="ps_m", bufs=1, space="PSUM") as ps_m, \
                 tc.tile_pool(name="ps_n", bufs=1, space="PSUM") as ps_n, \
                 tc.tile_pool(name="ps_t", bufs=2, space="PSUM") as ps_t:
                for k in range(nchunk):
                    src_s = eb.tile([128, cfg.T], i32, tag="src")
                    nc.sync.dma_start(out=src_s[:], in_=src_d[ts(k, 128), :])
                    em_s = eb.tile([128, cfg.T + 1], i32, tag="em")
                    nc.sync.dma_start(out=em_s[:], in_=em_d[ts(k, 128), :])
                    dst_s = eb.tile([128, cfg.T], i32, tag="dst")
                    nc.vector.tensor_scalar(dst_s[:], em_s[:, 0:cfg.T], 8191,
                                            None, op0=ALU.bitwise_and)
                    dlp_s = eb.tile([128, cfg.T], i32, tag="dlp")
                    nc.vector.tensor_scalar(dlp_s[:], em_s[:, 0:cfg.T], 13,
                                            None, op0=ALU.logical_shift_right)
                    dl_s = eb.tile([128, cfg.T], f32, tag="dl")
                    nc.vector.tensor_copy(dl_s[:], dlp_s[:])
                    pd_s = eb.tile([DIM, cfg.CHUNK_E], f16, tag="pd")
                    nc.sync.dma_start(out=pd_s[:], in_=pd_d[ts(k, DIM), :])

                    acc_p = ps_acc.tile([128, 2 * C], f32, tag="acc")

                    for b in range(NB):
                        esl = slice(b * BLK, (b + 1) * BLK)
                        # gathers for this block, one [128,1]-offset DMA per tile
                        vhgs, ugs = [], []
                        for tt in range(cfg.TB):
                            ti = b * cfg.TB + tt
                            vhg_t = ebg.tile([128, AH + C], f16, tag=f"vhg{tt}")
                            nc.gpsimd.indirect_dma_start(
                                out=vhg_t[:], out_offset=None, in_=vh_full[:],
                                in_offset=IndirectOffsetOnAxis(
                                    ap=src_s[:, ti:ti + 1], axis=0))
                            vhgs.append(vhg_t)
                            ug_t = ebg.tile([128, AH], f16, tag=f"ug{tt}")
                            nc.gpsimd.indirect_dma_start(
                                out=ug_t[:], out_offset=None, in_=U_loc[:],
                                in_offset=IndirectOffsetOnAxis(
                                    ap=dst_s[:, ti:ti + 1], axis=0))
                            ugs.append(ug_t)

                        # pos MLP
                        tp1_p = ps_m.tile([PH, BLK], f32, tag="tp1")
                        nc.tensor.matmul(tp1_p[:], lhsT=wp1_s,
                                         rhs=pd_s[:, esl], start=True, stop=True)
                        tp1_s = eb.tile([PH, BLK], f16, tag="tp1s")
                        nc.scalar.activation(tp1_s[:], tp1_p[:], AF.Relu, bias=bp1_b)
                        del_p = ps_b.tile([C, BLK], f32, tag="delp")
                        nc.tensor.matmul(del_p[:], lhsT=wp2_s,
                                         rhs=tp1_s[:], start=True, stop=True)
                        del_s = eb.tile([C, BLK], f16, tag="dels")
                        nc.scalar.activation(del_s[:], del_p[:], AF.Relu, bias=bp2_b)

                        # attn layer 1: z1 = Wa1^T delta + (U[dst]-V[src])^T.
                        # The per-tile gd transposes accumulate straight into
                        # the z1 PSUM group (PE executes in program order, so
                        # the start=True matmul lands first).
                        z1_p = ps_n.tile([AH, BLK], f32, tag="z1")
                        nc.tensor.matmul(z1_p[:], lhsT=wa1_s,
                                         rhs=del_s[:], start=True, stop=False)
                        for tt in range(cfg.TB):
                            gd_s = eb.tile([128, AH], f32, tag="gd")
                            nc.vector.tensor_tensor(gd_s[:], ugs[tt][:], vhgs[tt][:, 0:AH],
                                                    op=ALU.subtract)
                            csl = slice(tt * 128, (tt + 1) * 128)
                            nc.tensor.matmul(z1_p[:, csl], lhsT=gd_s[:], rhs=ident32_s[:],
                                             is_transpose=True, start=False, stop=True,
                                             skip_group_check=True)
                        ta_s = eb.tile([AH, BLK], f16, tag="ta")
                        nc.scalar.activation(ta_s[:], z1_p[:], AF.Relu, bias=ba1_b)

                        # attn layer 2 + exp
                        al_p = ps_c.tile([C, BLK], f32, tag="al")
                        nc.tensor.matmul(al_p[:], lhsT=wa2_s,
                                         rhs=ta_s[:], start=True, stop=True)
                        ar_s = eb.tile([C, BLK], f32, tag="ar")
                        nc.scalar.activation(ar_s[:], al_p[:], AF.Relu, bias=ba2_b)
                        e_s = eb.tile([C, BLK], f16, tag="e")
                        nc.scalar.activation(e_s[:], ar_s[:], AF.Exp, bias=shift_b)
                        ew2_s = eb.tile([C, BLK], f16, tag="ew2")
                        nc.vector.tensor_tensor(ew2_s[:], e_s[:], del_s[:], op=ALU.mult)

                        # per-tile: transpose, assemble [ew | e]^T, indicator, seg-matmul
                        for tt in range(cfg.TB):
                            ti = b * cfg.TB + tt
                            csl = slice(tt * 128, (tt + 1) * 128)
                            eT_p = ps_t.tile([128, 128], f16, tag="tr")
                            nc.tensor.transpose(eT_p[:], e_s[:, csl], ident_s[:])
                            ew2T_p = ps_t.tile([128, 128], f16, tag="tr")
                            nc.tensor.transpose(ew2T_p[:], ew2_s[:, csl], ident_s[:])
                            ewe_s = eb.tile([128, 2 * C], f16, tag="ewe")
                            nc.vector.tensor_copy(ewe_s[:, C:], eT_p[:])
                            tmp_s = eb.tile([128, C], f16, tag="tmp")
                            nc.vector.tensor_tensor(tmp_s[:], eT_p[:], vhgs[tt][:, AH:],
                                                    op=ALU.mult)
                            nc.vector.tensor_tensor(ewe_s[:, 0:C], tmp_s[:], ew2T_p[:],
                                                    op=ALU.add)
                            ind_s = eb.tile([128, 128], f16, tag="ind")
                            nc.vector.tensor_scalar(ind_s[:], iota_s[:], dl_s[:, ti:ti + 1],
                                                    None, op0=ALU.is_equal)
                            nc.tensor.matmul(acc_p[:], lhsT=ind_s[:],
                                             rhs=ewe_s[:],
                                             start=(ti == 0), stop=(ti == cfg.T - 1))

                    # finalize chunk
                    sp_s = eb.tile([128, C], f32, tag="sp")
                    nc.vector.tensor_scalar_add(sp_s[:], acc_p[:, C:], cfg.EPS)
                    rp_s = eb.tile([128, C], f32, tag="rp")
                    nc.vector.reciprocal(rp_s[:], sp_s[:])
                    o_s = eb.tile([128, C], f32, tag="o")
                    nc.vector.tensor_tensor(o_s[:], acc_p[:, 0:C], rp_s[:], op=ALU.mult)
                    o2_s = eb.tile([128, C], f16, tag="o2")
                    nc.scalar.activation(o2_s[:], o_s[:], AF.Relu)
                    nc.gpsimd.indirect_dma_start(
                        out=y_d[:], out_offset=IndirectOffsetOnAxis(
                            ap=em_s[:, cfg.T:cfg.T + 1], axis=0),
                        in_=o2_s[:], in_offset=None)
    nc.finalize()
    return nc


def _build_inputs(inputs, cfg):
    x = np.asarray(inputs["x"], np.float32)
    pos = np.ascontiguousarray(np.asarray(inputs["pos"], np.float32))
    W_lin = np.asarray(inputs["W_lin"], np.float32)
    W_src = np.asarray(inputs["W_src"], np.float32)
    W_dst = np.asarray(inputs["W_dst"], np.float32)
    Wp1 = np.asarray(inputs["Wp1"], np.float32)
    bp1 = np.asarray(inputs["bp1"], np.float32)
    Wp2 = np.asarray(inputs["Wp2"], np.float32)
    bp2 = np.asarray(inputs["bp2"], np.float32)
    Wa1 = np.asarray(inputs["Wa1"], np.float32)
    ba1 = np.asarray(inputs["ba1"], np.float32)
    Wa2 = np.asarray(inputs["Wa2"], np.float32)
    ba2 = np.asarray(inputs["ba2"], np.float32)

    Wda = (W_dst @ Wa1).astype(np.float16)   # [C, AH]
    Wsa = (W_src @ Wa1).astype(np.float16)
    wpack = np.zeros((128, WCOLS), np.float16)
    wpack[:, WC_NODE:WC_NODE + 256] = np.concatenate(
        [Wda, Wsa, W_lin.astype(np.float16)], axis=1)
    wpack[0:cfg.DIM, WC_P1:WC_P1 + cfg.PH] = Wp1
    wpack[0:cfg.PH, WC_P2:WC_P2 + cfg.C] = Wp2
    wpack[:, WC_A1:WC_A1 + cfg.AH] = Wa1
    wpack[0:cfg.AH, WC_A2:WC_A2 + cfg.C] = Wa2
    wpack[0:cfg.PH, WC_B + 0] = bp1
    wpack[0:cfg.C, WC_B + 1] = bp2
    wpack[0:cfg.AH, WC_B + 2] = ba1
    wpack[0:cfg.C, WC_B + 3] = ba2
    wpack[:, WC_B + 4] = -cfg.SHIFT

    packs, nchunk = _pack(x, pos, inputs["edge_index"], cfg)
    xh = x.astype(np.float16)
    in_maps = []
    for c, p in enumerate(packs):
        xT_c = np.ascontiguousarray(xh[c * cfg.NLOC:(c + 1) * cfg.NLOC, :].T)
        in_maps.append(dict(
            xT=xT_c, wpack=wpack,
            srcid=p["srcid"].reshape(-1, cfg.T),
            emap=p["emap"].reshape(-1, cfg.T + 1),
            posdT=p["posdT"].reshape(-1, cfg.CHUNK_E),
        ))
    return in_maps, nchunk


def kernel(**inputs):
    cfg = CFG
    in_maps, nchunk = _build_inputs(inputs, cfg)
    nc = _build(cfg, nchunk)
    res = run_bass_kernel_spmd(nc, in_maps, list(range(cfg.M)))
    y = np.concatenate([res.results[c]["y"][: cfg.NLOC] for c in range(cfg.M)], axis=0)
    return y.astype(np.float32)


# revision 12
# speedup vs baseline: 1.7822x; 1.7822x over previous
"""Trainium2 Bass kernel for nn_ClusterEncoder (PointTransformerConv-style
GNN message passing), 8-core SPMD.

Strategy (edges sharded by destination node; fp16 data plane):
  * Host: sort edges by dst, split nodes into 8 equal contiguous ranges
    (edge counts balance to ~0.3% for this random graph). Within a core,
    greedy-pack destination nodes into "chunks" of <=128 nodes and
    <=CHUNK_E edges; pad each chunk's edge list to CHUNK_E slots.
    Each core receives ONLY its node shard (xT, fp16, transposed) plus
    its edge maps, consolidated into 5 arrays (~3.3 MB/core instead of
    a replicated 25.6 MB x): per-edge dst ids, local chunk-row ids and
    per-chunk output rows ride in one packed int32 array.
  * Device, phase 1 (local shard only): U_loc = x_c @ (W_dst@Wa1) and
    VH_loc = x_c @ [W_src@Wa1 | W_lin] for the core's own 6250 nodes.
  * AllGather VH_loc across the 8 cores -> vh_full [N, 192] fp16
    (contiguous node shards concatenate rank-major, so global src ids
    index it directly). U stays local: dst ids are core-local by the
    edge sharding, so the U gather reads the local table.
  * Device, phase 2 (per chunk of 16 x 128-edge tiles):
      - gather VH rows by src (384B/row) and U rows by local dst,
      - pos MLP: t_p1 = relu(Wp1^T posd^T + bp1), delta = relu(Wp2^T t_p1 + bp2),
      - z1 = Wa1^T delta;  t_a = relu(z1 + (U[dst]-V[src])^T + ba1),
        with the per-tile (U-V)^T transposes accumulated straight into
        the z1 PSUM group,
      - logits = relu(Wa2^T t_a + ba2);  e = exp(logits - SHIFT)
        (softmax max-subtraction replaced by a constant shift -- exactly
        equivalent math since the shift cancels in e/sum(e); logits are
        relu-bounded so no overflow),
      - one-hot indicator per tile from local dst index (iota + is_equal),
      - segment-sum via matmul: acc[n, 0:128] += ind^T @ (e*(H[src]+delta))^T,
        acc[n, 128:256] += ind^T @ e^T   (numerator and normalizer together),
      - out = relu(NUM / (s + eps)); indirect-scatter fp16 rows to y.
  * Softmax segments are core-local by construction, so the only
    collective is the single VH AllGather.
"""
import sys
from dataclasses import dataclass
from math import ceil

if "/opt/trn_rl_repo" not in sys.path:
    sys.path.insert(0, "/opt/trn_rl_repo")

import numpy as np

import concourse.bass as bass
import concourse.mybir as mybir
import concourse.tile as tile
from concourse import bacc
from concourse.bass import IndirectOffsetOnAxis, ts
from concourse.bass_utils import run_bass_kernel_spmd
from concourse.masks import make_identity

f32 = mybir.dt.float32
f16 = mybir.dt.float16
i32 = mybir.dt.int32
AF = mybir.ActivationFunctionType
ALU = mybir.AluOpType


@dataclass
class Cfg:
    N: int = 50000
    C: int = 128
    PH: int = 64
    AH: int = 64
    DIM: int = 2
    M: int = 8            # cores
    T: int = 16           # 128-edge tiles per chunk
    TB: int = 4           # tiles per matmul block (block = 512 edges)
    SHIFT: float = 8.0
    EPS: float = 1e-12

    @property
    def NLOC(self):
        return self.N // self.M

    @property
    def CHUNK_E(self):
        return self.T * 128

    @property
    def OUT_ROWS(self):
        return self.NLOC + 1  # +1 trash row for padded scatter lanes


CFG = Cfg()

# wpack column layout (fp16 [128, WCOLS])
WC_NODE = 0          # [0:128, 0:256]   Wda | Wsa | W_lin
WC_P1 = 256          # [0:2,   256:320] Wp1
WC_P2 = 320          # [0:64,  320:448] Wp2
WC_A1 = 448          # [0:128, 448:512] Wa1
WC_A2 = 512          # [0:64,  512:640] Wa2
WC_B = 640           # [0:128, 640:645] bp1 | bp2 | ba1 | ba2 | -SHIFT
WCOLS = 648


# ---------------------------------------------------------------- host pack
def _pack(x, pos, edge_index, cfg):
    """Sort/shard/chunk edges; returns per-core input dicts (minus weights)."""
    src = np.asarray(edge_index[0], np.int64)
    dst = np.asarray(edge_index[1], np.int64)
    order = np.argsort(dst, kind="stable")
    s_s = src[order]
    d_s = dst[order]
    posd = (pos[d_s] - pos[s_s]).astype(np.float16)  # [E, 2]

    NLOC = cfg.NLOC
    bounds = np.searchsorted(d_s, np.arange(cfg.M + 1) * NLOC)

    cores = []
    for c in range(cfg.M):
        lo, hi = bounds[c], bounds[c + 1]
        dloc = d_s[lo:hi] - c * NLOC
        deg = np.bincount(dloc, minlength=NLOC)
        nodes = np.nonzero(deg)[0]
        chunks = []  # (node_list, e0, e1) ; e relative to lo
        cur, cur_e, estart = [], 0, 0
        for n in nodes:
            dn = int(deg[n])
            assert dn <= cfg.CHUNK_E, f"degree {dn} exceeds chunk capacity"
            if len(cur) == 128 or cur_e + dn > cfg.CHUNK_E:
                chunks.append((cur, estart, estart + cur_e))
                estart += cur_e
                cur, cur_e = [], 0
            cur.append(int(n))
            cur_e += dn
        if cur:
            chunks.append((cur, estart, estart + cur_e))
        cores.append((lo, chunks, dloc))

    NCHUNK = max(len(ch) for _, ch, _ in cores) if cores else 1
    NCHUNK = max(NCHUNK, 1)

    in_maps = []
    for c in range(cfg.M):
        lo, chunks, dloc = cores[c]
        srcid = np.zeros((NCHUNK, 128, cfg.T), np.int32)
        # emap[..., :T] = local dst id | (local chunk row + 1) << 13
        # emap[..., T]  = per-chunk output rows (trash row NLOC for pads)
        emap = np.zeros((NCHUNK, 128, cfg.T + 1), np.int32)
        emap[:, :, cfg.T] = cfg.NLOC
        posdT = np.zeros((NCHUNK, cfg.DIM, cfg.CHUNK_E), np.float16)
        for k, (nl, e0, e1) in enumerate(chunks):
            cnt = e1 - e0
            g0, g1 = lo + e0, lo + e1
            nla = np.asarray(nl, np.int64)
            loc = np.searchsorted(nla, dloc[e0:e1]).astype(np.int32)
            j = np.arange(cnt)
            t_idx = j >> 7
            lane = j & 127
            srcid[k, lane, t_idx] = s_s[g0:g1].astype(np.int32)
            emap[k, lane, t_idx] = (dloc[e0:e1].astype(np.int32)
                                    | ((loc + 1) << 13))
            posdT[k, :, :cnt] = posd[g0:g1].T
            emap[k, : len(nl), cfg.T] = nla.astype(np.int32)
        in_maps.append(dict(srcid=srcid, emap=emap, posdT=posdT))
    return in_maps, NCHUNK


# ---------------------------------------------------------------- program
def _build(cfg, nchunk):
    nc = bacc.Bacc(None, target_bir_lowering=False, num_devices=cfg.M)
    N, C, PH, AH, DIM = cfg.N, cfg.C, cfg.PH, cfg.AH, cfg.DIM
    NLOC = cfg.NLOC

    xT_d = nc.declare_dram_parameter("xT", [C, NLOC], f16, isOutput=False)
    wpack_d = nc.declare_dram_parameter("wpack", [128, WCOLS], f16, isOutput=False)
    src_d = nc.declare_dram_parameter("srcid", [nchunk * 128, cfg.T], i32, isOutput=False)
    em_d = nc.declare_dram_parameter("emap", [nchunk * 128, cfg.T + 1], i32, isOutput=False)
    pd_d = nc.declare_dram_parameter("posdT", [nchunk * DIM, cfg.CHUNK_E], f16, isOutput=False)
    y_d = nc.declare_dram_parameter("y", [cfg.OUT_ROWS, C], f16, isOutput=True)

    U_loc = nc.dram_tensor("U_loc", [cfg.OUT_ROWS, AH], f16)  # x_c @ (W_dst@Wa1)
    vh_send = nc.dram_tensor("vh_send", [NLOC, AH + C], f16)  # x_c @ [W_src@Wa1 | W_lin]
    vh_full = nc.dram_tensor("vh_full", [N, AH + C], f16, addr_space="Shared")

    NB = cfg.T // cfg.TB  # blocks per chunk
    BLK = cfg.TB * 128

    with tile.TileContext(nc) as tc:
        with tc.tile_pool(name="const", bufs=1) as cp:
            wpack_s = cp.tile([128, WCOLS], f16)
            nc.sync.dma_start(out=wpack_s[:], in_=wpack_d[:, :])
            wnode_s = wpack_s[:, WC_NODE:WC_NODE + 2 * AH + C]
            wp1_s = wpack_s[0:DIM, WC_P1:WC_P1 + PH]
            wp2_s = wpack_s[0:PH, WC_P2:WC_P2 + C]
            wa1_s = wpack_s[:, WC_A1:WC_A1 + AH]
            wa2_s = wpack_s[0:AH, WC_A2:WC_A2 + C]
            bp1_b = wpack_s[0:PH, WC_B + 0:WC_B + 1]
            bp2_b = wpack_s[:, WC_B + 1:WC_B + 2]
            ba1_b = wpack_s[0:AH, WC_B + 2:WC_B + 3]
            ba2_b = wpack_s[:, WC_B + 3:WC_B + 4]
            shift_b = wpack_s[:, WC_B + 4:WC_B + 5]
            ident_s = cp.tile([128, 128], f16)
            make_identity(nc, ident_s[:])
            ident32_s = cp.tile([128, 128], f32)
            make_identity(nc, ident32_s[:])
            iota_i = cp.tile([128, 128], i32)
            nc.gpsimd.iota(iota_i[:], pattern=[[1, 128]], base=1, channel_multiplier=0)
            iota_s = cp.tile([128, 128], f16)
            nc.vector.tensor_copy(iota_s[:], iota_i[:])

            # ---------------- phase 1: local node features U / VH ----------
            with tc.tile_pool(name="p1", bufs=3) as p1, \
                 tc.tile_pool(name="p1ps", bufs=2, space="PSUM") as p1ps:
                zr_s = p1.tile([1, AH], f16, tag="zr")
                nc.gpsimd.memset(zr_s[:], 0.0)
                nc.sync.dma_start(out=U_loc[NLOC:NLOC + 1, :], in_=zr_s[:])

                def p1_body(xsl, usl, rows):
                    # lhsT must sit at a static offset (no register offsets
                    # in ldweights), so DMA each xT tile instead of slicing.
                    xt_s = p1.tile([C, 128], f16, tag="xt")
                    nc.sync.dma_start(out=xt_s[:, :rows], in_=xT_d[:, xsl])
                    uvh_p = p1ps.tile([128, 2 * AH + C], f32, tag="uvh")
                    nc.tensor.matmul(uvh_p[:rows, :], lhsT=xt_s[:, :rows],
                                     rhs=wnode_s, start=True, stop=True)
                    uvh_s = p1.tile([128, 2 * AH + C], f16, tag="uvhs")
                    nc.scalar.activation(uvh_s[:rows, :], uvh_p[:rows, :], AF.Copy)
                    nc.sync.dma_start(out=U_loc[usl, :], in_=uvh_s[:rows, 0:AH])
                    nc.sync.dma_start(out=vh_send[usl, :], in_=uvh_s[:rows, AH:])

                nfull = NLOC // 128
                tc.For_i_unrolled(
                    0, nfull, 1,
                    lambda t: p1_body(ts(t, 128), ts(t, 128), 128),
                    max_unroll=8)
                if NLOC % 128:
                    p1_body(slice(nfull * 128, NLOC), slice(nfull * 128, NLOC),
                            NLOC % 128)

            # ---------------- all-gather VH across cores ----------
            nc.gpsimd.collective_compute(
                "AllGather",
                mybir.AluOpType.bypass,
                replica_groups=[list(range(cfg.M))],
                ins=[vh_send[:, :]],
                outs=[vh_full[:, :]],
            )

            # ---------------- phase 2: edges ----------------
            with tc.tile_pool(name="eb", bufs=3) as eb, \
                 tc.tile_pool(name="ebg", bufs=3) as ebg, \
                 tc.tile_pool(name="ps_acc", bufs=2, space="PSUM") as ps_acc, \
                 tc.tile_pool(name="ps_b", bufs=1, space="PSUM") as ps_b, \
                 tc.tile_pool(name="ps_c", bufs=1, space="PSUM") as ps_c, \
                 tc.tile_pool(name="ps_m", bufs=1, space="PSUM") as ps_m, \
                 tc.tile_pool(name="ps_n", bufs=1, space="PSUM") as ps_n, \
                 tc.tile_pool(name="ps_t", bufs=2, space="PSUM") as ps_t:
                def chunk_body(k):
                    src_s = eb.tile([128, cfg.T], i32, tag="src")
                    nc.sync.dma_start(out=src_s[:], in_=src_d[ts(k, 128), :])
                    em_s = eb.tile([128, cfg.T + 1], i32, tag="em")
                    nc.sync.dma_start(out=em_s[:], in_=em_d[ts(k, 128), :])
                    dst_s = eb.tile([128, cfg.T], i32, tag="dst")
                    nc.vector.tensor_scalar(dst_s[:], em_s[:, 0:cfg.T], 8191,
                                            None, op0=ALU.bitwise_and)
                    dlp_s = eb.tile([128, cfg.T], i32, tag="dlp")
                    nc.vector.tensor_scalar(dlp_s[:], em_s[:, 0:cfg.T], 13,
                                            None, op0=ALU.logical_shift_right)
                    dl_s = eb.tile([128, cfg.T], f32, tag="dl")
                    nc.vector.tensor_copy(dl_s[:], dlp_s[:])
                    pd_s = eb.tile([DIM, cfg.CHUNK_E], f16, tag="pd")
                    nc.sync.dma_start(out=pd_s[:], in_=pd_d[ts(k, DIM), :])

                    acc_p = ps_acc.tile([128, 2 * C], f32, tag="acc")

                    for b in range(NB):
                        esl = slice(b * BLK, (b + 1) * BLK)
                        # gathers for this block, one [128,1]-offset DMA per tile
                        vhgs, ugs = [], []
                        for tt in range(cfg.TB):
                            ti = b * cfg.TB + tt
                            vhg_t = ebg.tile([128, AH + C], f16, tag=f"vhg{tt}")
                            nc.gpsimd.indirect_dma_start(
                                out=vhg_t[:], out_offset=None, in_=vh_full[:],
                                in_offset=IndirectOffsetOnAxis(
                                    ap=src_s[:, ti:ti + 1], axis=0))
                            vhgs.append(vhg_t)
                            ug_t = ebg.tile([128, AH], f16, tag=f"ug{tt}")
                            nc.gpsimd.indirect_dma_start(
                                out=ug_t[:], out_offset=None, in_=U_loc[:],
                                in_offset=IndirectOffsetOnAxis(
                                    ap=dst_s[:, ti:ti + 1], axis=0))
                            ugs.append(ug_t)

                        # pos MLP
                        tp1_p = ps_m.tile([PH, BLK], f32, tag="tp1")
                        nc.tensor.matmul(tp1_p[:], lhsT=wp1_s,
                                         rhs=pd_s[:, esl], start=True, stop=True)
                        tp1_s = eb.tile([PH, BLK], f16, tag="tp1s")
                        nc.scalar.activation(tp1_s[:], tp1_p[:], AF.Relu, bias=bp1_b)
                        del_p = ps_b.tile([C, BLK], f32, tag="delp")
                        nc.tensor.matmul(del_p[:], lhsT=wp2_s,
                                         rhs=tp1_s[:], start=True, stop=True)
                        del_s = eb.tile([C, BLK], f16, tag="dels")
                        nc.scalar.activation(del_s[:], del_p[:], AF.Relu, bias=bp2_b)

                        # attn layer 1: z1 = Wa1^T delta + (U[dst]-V[src])^T.
                        # The per-tile gd transposes accumulate straight into
                        # the z1 PSUM group (PE executes in program order, so
                        # the start=True matmul lands first).
                        z1_p = ps_n.tile([AH, BLK], f32, tag="z1")
                        nc.tensor.matmul(z1_p[:], lhsT=wa1_s,
                                         rhs=del_s[:], start=True, stop=False)
                        for tt in range(cfg.TB):
                            gd_s = eb.tile([128, AH], f32, tag="gd")
                            nc.vector.tensor_tensor(gd_s[:], ugs[tt][:], vhgs[tt][:, 0:AH],
                                                    op=ALU.subtract)
                            csl = slice(tt * 128, (tt + 1) * 128)
                            nc.tensor.matmul(z1_p[:, csl], lhsT=gd_s[:], rhs=ident32_s[:],
                                             is_transpose=True, start=False, stop=True,
                                             skip_group_check=True)
                        ta_s = eb.tile([AH, BLK], f16, tag="ta")
                        nc.scalar.activation(ta_s[:], z1_p[:], AF.Relu, bias=ba1_b)

                        # attn layer 2 + exp
                        al_p = ps_c.tile([C, BLK], f32, tag="al")
                        nc.tensor.matmul(al_p[:], lhsT=wa2_s,
                                         rhs=ta_s[:], start=True, stop=True)
                        ar_s = eb.tile([C, BLK], f32, tag="ar")
                        nc.scalar.activation(ar_s[:], al_p[:], AF.Relu, bias=ba2_b)
                        e_s = eb.tile([C, BLK], f16, tag="e")
                        nc.scalar.activation(e_s[:], ar_s[:], AF.Exp, bias=shift_b)
                        ew2_s = eb.tile([C, BLK], f16, tag="ew2")
                        nc.vector.tensor_tensor(ew2_s[:], e_s[:], del_s[:], op=ALU.mult)

                        # per-tile: transpose, assemble [ew | e]^T, indicator, seg-matmul
                        for tt in range(cfg.TB):
                            ti = b * cfg.TB + tt
                            csl = slice(tt * 128, (tt + 1) * 128)
                            eT_p = ps_t.tile([128, 128], f16, tag="tr")
                            nc.tensor.transpose(eT_p[:], e_s[:, csl], ident_s[:])
                            ew2T_p = ps_t.tile([128, 128], f16, tag="tr")
                            nc.tensor.transpose(ew2T_p[:], ew2_s[:, csl], ident_s[:])
                            ewe_s = eb.tile([128, 2 * C], f16, tag="ewe")
                            nc.vector.tensor_copy(ewe_s[:, C:], eT_p[:])
                            tmp_s = eb.tile([128, C], f16, tag="tmp")
                            nc.vector.tensor_tensor(tmp_s[:], eT_p[:], vhgs[tt][:, AH:],
                                                    op=ALU.mult)
                            nc.vector.tensor_tensor(ewe_s[:, 0:C], tmp_s[:], ew2T_p[:],
                                                    op=ALU.add)
                            ind_s = eb.tile([128, 128], f16, tag="ind")
                            nc.vector.tensor_scalar(ind_s[:], iota_s[:], dl_s[:, ti:ti + 1],
                                                    None, op0=ALU.is_equal)
                            nc.tensor.matmul(acc_p[:], lhsT=ind_s[:],
                                             rhs=ewe_s[:],
                                             start=(ti == 0), stop=(ti == cfg.T - 1))

                    # finalize chunk
                    sp_s = eb.tile([128, C], f32, tag="sp")
                    nc.vector.tensor_scalar_add(sp_s[:], acc_p[:, C:], cfg.EPS)
                    rp_s = eb.tile([128, C], f32, tag="rp")
                    nc.vector.reciprocal(rp_s[:], sp_s[:])
                    o_s = eb.tile([128, C], f32, tag="o")
                    nc.vector.tensor_tensor(o_s[:], acc_p[:, 0:C], rp_s[:], op=ALU.mult)
                    o2_s = eb.tile([128, C], f16, tag="o2")
                    nc.scalar.activation(o2_s[:], o_s[:], AF.Relu)
                    nc.gpsimd.indirect_dma_start(
                        out=y_d[:], out_offset=IndirectOffsetOnAxis(
                            ap=em_s[:, cfg.T:cfg.T + 1], axis=0),
                        in_=o2_s[:], in_offset=None)

                tc.For_i_unrolled(0, nchunk, 1, chunk_body, max_unroll=4)
    nc.finalize()
    return nc


def _build_inputs(inputs, cfg):
    x = np.asarray(inputs["x"], np.float32)
    pos = np.ascontiguousarray(np.asarray(inputs["pos"], np.float32))
    W_lin = np.asarray(inputs["W_lin"], np.float32)
    W_src = np.asarray(inputs["W_src"], np.float32)
    W_dst = np.asarray(inputs["W_dst"], np.float32)
    Wp1 = np.asarray(inputs["Wp1"], np.float32)
    bp1 = np.asarray(inputs["bp1"], np.float32)
    Wp2 = np.asarray(inputs["Wp2"], np.float32)
    bp2 = np.asarray(inputs["bp2"], np.float32)
    Wa1 = np.asarray(inputs["Wa1"], np.float32)
    ba1 = np.asarray(inputs["ba1"], np.float32)
    Wa2 = np.asarray(inputs["Wa2"], np.float32)
    ba2 = np.asarray(inputs["ba2"], np.float32)

    Wda = (W_dst @ Wa1).astype(np.float16)   # [C, AH]
    Wsa = (W_src @ Wa1).astype(np.float16)
    wpack = np.zeros((128, WCOLS), np.float16)
    wpack[:, WC_NODE:WC_NODE + 256] = np.concatenate(
        [Wda, Wsa, W_lin.astype(np.float16)], axis=1)
    wpack[0:cfg.DIM, WC_P1:WC_P1 + cfg.PH] = Wp1
    wpack[0:cfg.PH, WC_P2:WC_P2 + cfg.C] = Wp2
    wpack[:, WC_A1:WC_A1 + cfg.AH] = Wa1
    wpack[0:cfg.AH, WC_A2:WC_A2 + cfg.C] = Wa2
    wpack[0:cfg.PH, WC_B + 0] = bp1
    wpack[0:cfg.C, WC_B + 1] = bp2
    wpack[0:cfg.AH, WC_B + 2] = ba1
    wpack[0:cfg.C, WC_B + 3] = ba2
    wpack[:, WC_B + 4] = -cfg.SHIFT

    packs, nchunk = _pack(x, pos, inputs["edge_index"], cfg)
    xh = x.astype(np.float16)
    in_maps = []
    for c, p in enumerate(packs):
        xT_c = np.ascontiguousarray(xh[c * cfg.NLOC:(c + 1) * cfg.NLOC, :].T)
        in_maps.append(dict(
            xT=xT_c, wpack=wpack,
            srcid=p["srcid"].reshape(-1, cfg.T),
            emap=p["emap"].reshape(-1, cfg.T + 1),
            posdT=p["posdT"].reshape(-1, cfg.CHUNK_E),
        ))
    return in_maps, nchunk


def kernel(**inputs):
    cfg = CFG
    in_maps, nchunk = _build_inputs(inputs, cfg)
    nc = _build(cfg, nchunk)
    res = run_bass_kernel_spmd(nc, in_maps, list(range(cfg.M)))
    y = np.concatenate([res.results[c]["y"][: cfg.NLOC] for c in range(cfg.M)], axis=0)
    return y.astype(np.float32)
